# revision 24
# baseline (speedup 1.0000x reference)
"""Trainium2 Bass kernel for nn_PointTrans_Layer_up (knn_interpolate upsample).

Strategy (8 NeuronCores): core c owns (batch b = c//2, query-half h = c%2)
-> 8192 queries vs the batch's 4096 coarse points.

Per core:
  - h1 = x1 @ W1.T + b1 on PE (fp32r), staged to DRAM as the gather table.
  - Selection: s' = 2*y.p - ||p||^2 via an augmented matmul on PE
    (argmax_j s' == argmin_j d2).  DVE max/max_index give the exact top-8
    values + indices per query (matches jax top_k tie semantics).
  - Weights: d2 = ||y||^2 - s'_top8 clipped at 1e-16, w = 1/d2, normalized.
  - Interp: dma_gather fetches the 65536 h1 rows; ACT scales each row by
    its normalized weight; a constant block-mask matmul (fp32r) sums each
    query's 8 rows in PSUM.
"""
import sys

sys.path.insert(0, "/opt/trn_rl_repo")

import numpy as np

import concourse.bacc as bacc
import concourse.mybir as mybir
from concourse.bass_utils import run_bass_kernel_spmd
from concourse.tile import TileContext
from concourse.masks import make_identity

B, N1, N2, C = 4, 4096, 16384, 256
NQ = N2 // 2              # queries per core
NT = NQ // 128            # 64 query tiles per core
NJT = N1 // 128           # 32 coarse tiles
EPS = 1e-16

# selection matmul mode: "fp32" (4 cyc/row, exact fp32) or "bf16x3"
# (1 cyc/row, fp32-faithful via 6-term split products)
SEL_MODE = "bf16x3"

F32 = mybir.dt.float32
F32R = mybir.dt.float32r
BF16 = mybir.dt.bfloat16
FP16 = mybir.dt.float16
U16 = mybir.dt.uint16
I16 = mybir.dt.int16


def _split3(x):
    """Split fp32 array into three bf16 parts a+b+c ~= x."""
    import ml_dtypes
    a = x.astype(ml_dtypes.bfloat16)
    r1 = (x - a.astype(np.float32)).astype(np.float32)
    b = r1.astype(ml_dtypes.bfloat16)
    r2 = (r1 - b.astype(np.float32)).astype(np.float32)
    c = r2.astype(ml_dtypes.bfloat16)
    return a, b, c


def _sel_tables(y, p1, pn2):
    """Build lhsT/rhs K-row tables for s' = 2*y.p - pn2.

    Returns (yaug [K, NQ], paug [K, N1], dtype).
    """
    if SEL_MODE == "fp32":
        yaug = np.empty((4, y.shape[0]), np.float32)
        yaug[0:3] = (2.0 * y).T
        yaug[3] = 1.0
        paug = np.empty((4, p1.shape[0]), np.float32)
        paug[0:3] = p1.T
        paug[3] = -pn2
        return yaug, paug, F32
    # bf16x3: per coordinate keep products aa, ab, ba, ac, ca, bb
    # (error ~2^-27); pn2 via rows (1, -pn2{a,b,c}).
    import ml_dtypes
    y2 = (2.0 * y).astype(np.float32)
    ya, yb, yc = _split3(y2.T)          # [3, NQ] each
    pa, pb, pc = _split3(p1.T)          # [3, N1]
    na, nb, nc_ = _split3(-pn2[None, :])  # [1, N1]
    one = np.ones_like(ya[0:1])
    yrows, prows = [], []
    for i in range(3):
        for yr, pr in ((ya, pa), (ya, pb), (yb, pa), (ya, pc), (yc, pa),
                       (yb, pb)):
            yrows.append(yr[i:i + 1])
            prows.append(pr[i:i + 1])
    for nr in (na, nb, nc_):
        yrows.append(one)
        prows.append(nr)
    yaug = np.concatenate(yrows, 0).astype(ml_dtypes.bfloat16)
    paug = np.concatenate(prows, 0).astype(ml_dtypes.bfloat16)
    return yaug, paug, BF16


_NC_CACHE = {}


def build_nc(sel_k, sel_dt):
    key = (sel_k, sel_dt)
    if key in _NC_CACHE:
        return _NC_CACHE[key]
    nc = bacc.Bacc()
    YAUG = nc.declare_dram_parameter("yaug", [sel_k, NQ], sel_dt, isOutput=False)
    PAUG = nc.declare_dram_parameter("paug", [sel_k, N1], sel_dt, isOutput=False)
    QN2T = nc.declare_dram_parameter("qn2t", [128, NT], F32, isOutput=False)
    X1T = nc.declare_dram_parameter("x1t", [2, 128, N1], F32, isOutput=False)
    W1T = nc.declare_dram_parameter("w1t", [2, 128, C], F32, isOutput=False)
    B1R = nc.declare_dram_parameter("b1rep", [128, C], F32, isOutput=False)
    MSK = nc.declare_dram_parameter("mask16", [128, 16], FP16, isOutput=False)
    OUT = nc.declare_dram_parameter("out", [NQ, C], F32, isOutput=True)

    with TileContext(nc) as tc:
        with tc.tile_pool(name="const", bufs=1) as const, \
             tc.tile_pool(name="work", bufs=2) as work, \
             tc.tile_pool(name="small", bufs=3) as small, \
             tc.tile_pool(name="dram", bufs=1, space="DRAM") as dpool, \
             tc.tile_pool(name="psel", bufs=2, space="PSUM") as psel, \
             tc.tile_pool(name="ptr", bufs=1, space="PSUM") as ptr, \
             tc.tile_pool(name="pout", bufs=2, space="PSUM") as pout:

            H1D = dpool.tile([N1, C], FP16)
            IDXST = dpool.tile([NT, 16, 64], U16)  # [t, r, q'*8+l] wrapped
            WNST = dpool.tile([NT, 128, 8], F32)    # [t, p=16l+r, q']

            yaug_f = const.tile([128, NQ], sel_dt)
            paug_f = const.tile([128, N1], sel_dt)
            yaug = yaug_f[0:sel_k]
            paug = paug_f[0:sel_k]
            qn2t = const.tile([128, NT], F32)
            b1r = const.tile([128, C], F32)
            msk = const.tile([128, 16], FP16)
            ident = const.tile([128, 128], F32)
            make_identity(nc, ident[:])
            nc.sync.dma_start(out=yaug, in_=YAUG[:])
            nc.sync.dma_start(out=paug, in_=PAUG[:])
            nc.sync.dma_start(out=qn2t[:], in_=QN2T[:])
            nc.sync.dma_start(out=b1r[:], in_=B1R[:])
            nc.sync.dma_start(out=msk[:], in_=MSK[:])

            # ---- h1 = x1 @ W1.T + b1 (fp32r matmul), staged to DRAM ----
            x1t = []
            w1t = []
            for i in range(2):
                x1ti = const.tile([128, N1], F32, tag=f"x1t{i}", name=f"x1t{i}")
                w1ti = const.tile([128, C], F32, tag=f"w1t{i}", name=f"w1t{i}")
                x1t.append(x1ti)
                w1t.append(w1ti)
            for i in range(2):
                nc.sync.dma_start(out=x1t[i][:], in_=X1T[i])
                nc.sync.dma_start(out=w1t[i][:], in_=W1T[i])
            for jt in range(NJT):
                ph = pout.tile([128, C], F32, tag="po", name="ph")
                for ci in range(2):
                    nc.tensor.matmul(
                        ph[:],
                        x1t[ci][:, jt * 128:(jt + 1) * 128],
                        w1t[ci][:],
                        start=(ci == 0), stop=(ci == 1),
                    )
                h1sb = work.tile([128, C], FP16, tag="h1sb")
                nc.vector.tensor_add(h1sb[:], ph[:], b1r[:])
                nc.sync.dma_start(out=H1D[jt * 128:(jt + 1) * 128, :], in_=h1sb[:])

            # ---- selection: 64 query tiles ----
            for t in range(NT):
                ssb = work.tile([128, N1], F32, tag="ssb")
                for ch in range(8):
                    ps = psel.tile([128, 512], F32)
                    nc.tensor.matmul(
                        ps[:],
                        yaug_f[0:sel_k, t * 128:(t + 1) * 128],
                        paug_f[0:sel_k, ch * 512:(ch + 1) * 512],
                        start=True, stop=True,
                    )
                    nc.scalar.copy(out=ssb[:, ch * 512:(ch + 1) * 512], in_=ps[:])
                mx = small.tile([128, 8], F32, tag="mx")
                mi = small.tile([128, 8], U16, tag="mi")
                nc.vector.max(out=mx[:], in_=ssb[:])
                nc.vector.max_index(out=mi[:], in_max=mx[:], in_values=ssb[:])
                # d2 = qn2 - s' (clipped), w = 1/d2, wn = w / sum(w)
                d2 = small.tile([128, 8], F32, tag="d2")
                nc.vector.tensor_scalar(
                    out=d2[:], in0=mx[:], scalar1=-1.0, scalar2=qn2t[:, t:t + 1],
                    op0=mybir.AluOpType.mult, op1=mybir.AluOpType.add,
                )
                nc.vector.tensor_scalar_max(d2[:], d2[:], EPS)
                w8 = small.tile([128, 8], F32, tag="w8")
                nc.vector.reciprocal(out=w8[:], in_=d2[:])
                sw = small.tile([128, 1], F32, tag="sw")
                nc.vector.tensor_reduce(
                    out=sw[:], in_=w8[:], axis=mybir.AxisListType.X,
                    op=mybir.AluOpType.add,
                )
                rsw = small.tile([128, 1], F32, tag="rsw")
                nc.vector.reciprocal(out=rsw[:], in_=sw[:])
                if t % 4 == 0:
                    wn4 = work.tile([128, 32], F32, tag="wn4")
                wn = wn4[:, (t % 4) * 8:(t % 4 + 1) * 8]
                nc.vector.tensor_scalar_mul(wn, w8[:], rsw[:, 0:1])
                # stage idx: wrapped layout [16, 64] is byte-identical to
                # a row-major copy of mi [128, 8] (thanks to the pi-perm)
                nc.sync.dma_start(
                    out=IDXST[t].rearrange("r (a l) -> r a l", a=8),
                    in_=mi[:])
                # every 4 tiles: transpose wn4 on PE and stage with one DMA.
                # wnT[(t4, l), pi] lands at t4*4096B + l*512B + pi*4B which is
                # exactly WNST[t][p=16l+r][q'] (16*32B == 512B).
                if t % 4 == 3:
                    pwt = ptr.tile([32, 128], F32, name="pwt")
                    nc.tensor.transpose(pwt[:], wn4[:], ident[:])
                    wnT = small.tile([32, 128], F32, tag="wnT")
                    nc.vector.tensor_copy(out=wnT[:], in_=pwt[:])
                    nc.scalar.dma_start(
                        out=WNST[t - 3:t + 1].rearrange(
                            "t (l r) q -> (t l) (r q)", l=8),
                        in_=wnT[:])

            # ---- interp: one gather per query tile (1024 idxs) ----
            # gather list order i = 128*q' + 16*l + r -> G partition 16l+r,
            # sub-chunk q'; query q_local = 16*q' + r
            for t in range(NT):
                idxw = work.tile([128, 64], I16, tag="idxw")
                idx_wr = IDXST[t].bitcast(I16).unsqueeze(0).broadcast_to(
                    [8, 16, 64])
                nc.sync.dma_start(out=idxw[:], in_=idx_wr)
                wnre = work.tile([128, 8], F32, tag="wnre")
                nc.scalar.dma_start(out=wnre[:], in_=WNST[t])
                gt = work.tile([128, 8, C], FP16, tag="G")
                nc.gpsimd.dma_gather(
                    out_ap=gt[:], in_ap=H1D[:], idxs_ap=idxw[:],
                    num_idxs=1024, num_idxs_reg=1024, elem_size=C,
                )
                # wcol[p, c, m] = msk[p, m] * wnre[p, c]
                wcol = work.tile([128, 8, 16], FP16, tag="wcol")
                nc.vector.tensor_tensor(
                    wcol[:],
                    msk[:].unsqueeze(1).broadcast_to([128, 8, 16]),
                    wnre[:].unsqueeze(2).broadcast_to([128, 8, 16]),
                    op=mybir.AluOpType.mult)
                osb = work.tile([16, 8, C], F32, tag="osb")
                for hj in range(2):
                    po = pout.tile([16, 4, C], F32)
                    for cc in range(4):
                        ci = hj * 4 + cc
                        nc.tensor.matmul(
                            po[:, cc, :],
                            wcol[:, ci, :],
                            gt[:, ci, :],
                            start=True, stop=True,
                        )
                    nc.scalar.copy(
                        out=osb[:, hj * 4:(hj + 1) * 4, :], in_=po[:])
                # rows q = 16*c + m  ->  osb[m, c, :]
                nc.sync.dma_start(
                    out=OUT[t * 128:(t + 1) * 128, :]
                    .rearrange("(c m) d -> m c d", m=16),
                    in_=osb[:],
                )

    nc.finalize()
    _NC_CACHE[key] = nc
    return nc


def _prep_core(x1b, pos1b, yb, W1, b1):
    """Host-side layout prep for one core. yb: [NQ, 3] queries."""
    pn2 = (pos1b.astype(np.float32) ** 2).sum(-1).astype(np.float32)
    qn2 = (yb.astype(np.float32) ** 2).sum(-1).astype(np.float32)
    # permute queries within each 128-tile: partition pi holds query
    # 16*(pi%8) + pi//8 (so staging DMAs have contiguous runs)
    pi = np.arange(128)
    q_of_part = 16 * (pi % 8) + pi // 8
    perm = (np.arange(NT)[:, None] * 128 + q_of_part[None, :]).reshape(-1)
    yb = yb[perm]
    qn2 = qn2[perm]
    yaug, paug, _ = _sel_tables(yb, pos1b, pn2)
    import ml_dtypes
    mask16 = np.zeros((128, 16), np.float16)
    for p in range(128):
        mask16[p, p % 16] = 1.0
    return {
        "yaug": np.ascontiguousarray(yaug),
        "paug": np.ascontiguousarray(paug),
        "qn2t": np.ascontiguousarray(qn2.reshape(NT, 128).T),
        "x1t": np.ascontiguousarray(x1b.T.reshape(2, 128, N1)),
        "w1t": np.ascontiguousarray(W1.T.reshape(2, 128, C)),
        "b1rep": np.ascontiguousarray(np.broadcast_to(b1, (128, C))),
        "mask16": mask16,
    }


def kernel(x1, pos1, x2, pos2, W1, b1, W2, b2):
    x1 = np.asarray(x1, np.float32)
    pos1 = np.asarray(pos1, np.float32)
    pos2 = np.asarray(pos2, np.float32)
    W1 = np.asarray(W1, np.float32)
    b1 = np.asarray(b1, np.float32)

    _, _, sel_dt = _sel_tables(pos2[0, :8, :], pos1[0, :8, :],
                               np.zeros(8, np.float32))
    sel_k = {"fp32": 4}.get(SEL_MODE, 21)
    nc = build_nc(sel_k, sel_dt)

    in_maps = []
    for c in range(8):
        b, h = c // 2, c % 2
        yb = pos2[b, h * NQ:(h + 1) * NQ]
        in_maps.append(_prep_core(x1[b], pos1[b], yb, W1, b1))

    import os
    trace = os.environ.get("KNN_TRACE", "0") == "1"
    if trace:
        import types, ctypes, contextlib
        import concourse.bass_utils as bu
        bu.upload_artifacts = lambda d: str(d)
        lib = ctypes.CDLL("/opt/axon/libaxon_pjrt.so")
        lib.axon_start_nrt_profile.argtypes = [
            ctypes.POINTER(ctypes.c_int64), ctypes.c_size_t]
        lib.axon_start_nrt_profile.restype = ctypes.c_int64
        lib.axon_stop_nrt_profile.argtypes = [ctypes.c_char_p]
        lib.axon_stop_nrt_profile.restype = ctypes.c_int64

        @contextlib.contextmanager
        def _hook(output_dir, device_ids):
            import jax
            jax.devices()
            if device_ids:
                ids = (ctypes.c_int64 * len(device_ids))(*device_ids)
                rc = lib.axon_start_nrt_profile(ids, len(device_ids))
            else:
                rc = lib.axon_start_nrt_profile(None, 0)
            if rc != 0:
                raise RuntimeError(f"axon_start_nrt_profile rc={rc}")
            try:
                yield
            finally:
                n = lib.axon_stop_nrt_profile(str(output_dir).encode())
                print(f"profile: {n} ntff file(s) written to {output_dir}")

        mod = types.ModuleType("antenv.axon_hooks")
        mod.get_axon_ntff_profile_hook = lambda: (
            lambda output_dir, device_ids: _hook(output_dir, device_ids))
        sys.modules["antenv.axon_hooks"] = mod
        res = run_bass_kernel_spmd(
            nc, in_maps, list(range(8)), trace=True,
            tmpdir=os.environ.get("KNN_TRACE_DIR") or None)
        print("HW exec time:", res.exec_time_ns, "ns")
        if res.mean_exec_time_ns:
            print("mean exec time:", res.mean_exec_time_ns, "ns")
    else:
        res = run_bass_kernel_spmd(nc, in_maps, list(range(8)))

    out = np.empty((B, N2, C), np.float32)
    for c in range(8):
        b, h = c // 2, c % 2
        out[b, h * NQ:(h + 1) * NQ] = res.results[c]["out"]
    return out


if __name__ == "__main__":
    rng = np.random.default_rng(0)
    ins = {
        "x1": rng.standard_normal((B, N1, C), np.float32),
        "pos1": rng.random((B, N1, 3), np.float32),
        "x2": rng.standard_normal((B, N2, C), np.float32),
        "pos2": rng.random((B, N2, 3), np.float32),
        "W1": (rng.standard_normal((C, C), np.float32) * 0.05).astype(np.float32),
        "b1": (rng.standard_normal((C,), np.float32) * 0.05).astype(np.float32),
        "W2": rng.standard_normal((C, C), np.float32),
        "b2": rng.standard_normal((C,), np.float32),
    }
    o = kernel(**ins)
    print("kernel out", o.shape, o.dtype, np.abs(o).max())


# revision 27
# speedup vs baseline: 1.0960x; 1.0960x over previous
"""Trainium2 Bass kernel for nn_PointTrans_Layer_up (knn_interpolate upsample).

Strategy (8 NeuronCores): core c owns (batch b = c//2, query-half h = c%2)
-> 8192 queries vs the batch's 4096 coarse points.

Per core:
  - h1 = x1 @ W1.T + b1 on PE (fp32r), staged to DRAM as the gather table.
  - Selection: s' = 2*y.p - ||p||^2 via an augmented matmul on PE
    (argmax_j s' == argmin_j d2).  DVE max/max_index give the exact top-8
    values + indices per query (matches jax top_k tie semantics).
  - Weights: d2 = ||y||^2 - s'_top8 clipped at 1e-16, w = 1/d2, normalized.
  - Interp: dma_gather fetches the 65536 h1 rows; ACT scales each row by
    its normalized weight; a constant block-mask matmul (fp32r) sums each
    query's 8 rows in PSUM.
"""
import sys

sys.path.insert(0, "/opt/trn_rl_repo")

import numpy as np

import concourse.bacc as bacc
import concourse.mybir as mybir
from concourse.bass_utils import run_bass_kernel_spmd
from concourse.tile import TileContext
from concourse.masks import make_identity

B, N1, N2, C = 4, 4096, 16384, 256
NQ = N2 // 2              # queries per core
NT = NQ // 128            # 64 query tiles per core
NJT = N1 // 128           # 32 coarse tiles
EPS = 1e-16

# selection matmul mode: "fp32" (4 cyc/row, exact fp32) or "bf16x3"
# (1 cyc/row, fp32-faithful via 6-term split products)
SEL_MODE = "bf16x3"

F32 = mybir.dt.float32
F32R = mybir.dt.float32r
BF16 = mybir.dt.bfloat16
FP16 = mybir.dt.float16
U16 = mybir.dt.uint16
I16 = mybir.dt.int16


def _split3(x):
    """Split fp32 array into three bf16 parts a+b+c ~= x."""
    import ml_dtypes
    a = x.astype(ml_dtypes.bfloat16)
    r1 = (x - a.astype(np.float32)).astype(np.float32)
    b = r1.astype(ml_dtypes.bfloat16)
    r2 = (r1 - b.astype(np.float32)).astype(np.float32)
    c = r2.astype(ml_dtypes.bfloat16)
    return a, b, c


def _sel_tables(y, p1, pn2):
    """Build lhsT/rhs K-row tables for s' = 2*y.p - pn2.

    Returns (yaug [K, NQ], paug [K, N1], dtype).
    """
    if SEL_MODE == "fp32":
        yaug = np.empty((4, y.shape[0]), np.float32)
        yaug[0:3] = (2.0 * y).T
        yaug[3] = 1.0
        paug = np.empty((4, p1.shape[0]), np.float32)
        paug[0:3] = p1.T
        paug[3] = -pn2
        return yaug, paug, F32
    # bf16x3: per coordinate keep products aa, ab, ba, ac, ca, bb
    # (error ~2^-27); pn2 via rows (1, -pn2{a,b,c}).
    import ml_dtypes
    y2 = (2.0 * y).astype(np.float32)
    ya, yb, yc = _split3(y2.T)          # [3, NQ] each
    pa, pb, pc = _split3(p1.T)          # [3, N1]
    na, nb, nc_ = _split3(-pn2[None, :])  # [1, N1]
    one = np.ones_like(ya[0:1])
    yrows, prows = [], []
    for i in range(3):
        for yr, pr in ((ya, pa), (ya, pb), (yb, pa), (ya, pc), (yc, pa),
                       (yb, pb)):
            yrows.append(yr[i:i + 1])
            prows.append(pr[i:i + 1])
    for nr in (na, nb, nc_):
        yrows.append(one)
        prows.append(nr)
    yaug = np.concatenate(yrows, 0).astype(ml_dtypes.bfloat16)
    paug = np.concatenate(prows, 0).astype(ml_dtypes.bfloat16)
    return yaug, paug, BF16


_NC_CACHE = {}


def build_nc(sel_k, sel_dt):
    key = (sel_k, sel_dt)
    if key in _NC_CACHE:
        return _NC_CACHE[key]
    nc = bacc.Bacc()
    YAUG = nc.declare_dram_parameter("yaug", [sel_k, NQ], sel_dt, isOutput=False)
    PAUG = nc.declare_dram_parameter("paug", [sel_k, N1], sel_dt, isOutput=False)
    QN2T = nc.declare_dram_parameter("qn2t", [128, NT], F32, isOutput=False)
    X1T = nc.declare_dram_parameter("x1t", [2, 128, N1], F32, isOutput=False)
    W1T = nc.declare_dram_parameter("w1t", [2, 128, C], F32, isOutput=False)
    B1R = nc.declare_dram_parameter("b1rep", [128, C], F32, isOutput=False)
    MSK = nc.declare_dram_parameter("mask16", [128, 16], FP16, isOutput=False)
    OUT = nc.declare_dram_parameter("out", [NQ, C], F32, isOutput=True)

    with TileContext(nc) as tc:
        with tc.tile_pool(name="const", bufs=1) as const, \
             tc.tile_pool(name="work", bufs=2) as work, \
             tc.tile_pool(name="small", bufs=3) as small, \
             tc.tile_pool(name="dram", bufs=1, space="DRAM") as dpool, \
             tc.tile_pool(name="psel", bufs=2, space="PSUM") as psel, \
             tc.tile_pool(name="ptr", bufs=1, space="PSUM") as ptr, \
             tc.tile_pool(name="pout", bufs=2, space="PSUM") as pout:

            H1D = dpool.tile([N1, C], FP16)
            IDXST = dpool.tile([NT, 16, 64], U16)  # [t, r, q'*8+l] wrapped
            WNST = dpool.tile([NT, 128, 8], F32)    # [t, p=16l+r, q']

            yaug_f = const.tile([128, NQ], sel_dt)
            paug_f = const.tile([128, N1], sel_dt)
            yaug = yaug_f[0:sel_k]
            paug = paug_f[0:sel_k]
            qn2t = const.tile([128, NT], F32)
            b1r = const.tile([128, C], F32)
            msk = const.tile([128, 16], FP16)
            ident = const.tile([128, 128], F32)
            make_identity(nc, ident[:])
            nc.sync.dma_start(out=yaug, in_=YAUG[:])
            nc.sync.dma_start(out=paug, in_=PAUG[:])
            nc.sync.dma_start(out=qn2t[:], in_=QN2T[:])
            nc.sync.dma_start(out=b1r[:], in_=B1R[:])
            nc.sync.dma_start(out=msk[:], in_=MSK[:])

            # ---- h1 = x1 @ W1.T + b1 (fp32r matmul), staged to DRAM ----
            x1t = []
            w1t = []
            for i in range(2):
                x1ti = const.tile([128, N1], F32, tag=f"x1t{i}", name=f"x1t{i}")
                w1ti = const.tile([128, C], F32, tag=f"w1t{i}", name=f"w1t{i}")
                x1t.append(x1ti)
                w1t.append(w1ti)
            for i in range(2):
                nc.sync.dma_start(out=x1t[i][:], in_=X1T[i])
                nc.sync.dma_start(out=w1t[i][:], in_=W1T[i])
            for jt in range(NJT):
                ph = pout.tile([128, C], F32, tag="po", name="ph")
                for ci in range(2):
                    nc.tensor.matmul(
                        ph[:],
                        x1t[ci][:, jt * 128:(jt + 1) * 128],
                        w1t[ci][:],
                        start=(ci == 0), stop=(ci == 1),
                    )
                h1sb = work.tile([128, C], FP16, tag="h1sb")
                nc.vector.tensor_add(h1sb[:], ph[:], b1r[:])
                nc.sync.dma_start(out=H1D[jt * 128:(jt + 1) * 128, :], in_=h1sb[:])

            # ---- selection: 64 query tiles ----
            for t in range(NT):
                ssb = work.tile([128, N1], F32, tag="ssb")
                for ch in range(8):
                    ps = psel.tile([128, 512], F32)
                    nc.tensor.matmul(
                        ps[:],
                        yaug_f[0:sel_k, t * 128:(t + 1) * 128],
                        paug_f[0:sel_k, ch * 512:(ch + 1) * 512],
                        start=True, stop=True,
                    )
                    nc.scalar.copy(out=ssb[:, ch * 512:(ch + 1) * 512], in_=ps[:])
                if t % 4 == 0:
                    mx4 = work.tile([128, 32], F32, tag="mx4")
                    wn4 = work.tile([128, 32], F32, tag="wn4")
                mx = mx4[:, (t % 4) * 8:(t % 4 + 1) * 8]
                mi = small.tile([128, 8], U16, tag="mi")
                nc.vector.max(out=mx, in_=ssb[:])
                nc.vector.max_index(out=mi[:], in_max=mx, in_values=ssb[:])
                if t % 4 == 3:
                    T4 = t - 3
                    # d2 = qn2 - s' (clipped), w = 1/d2, wn = w/sum(w),
                    # batched over 4 tiles
                    d24 = work.tile([128, 32], F32, tag="d24")
                    nc.vector.tensor_scalar_mul(d24[:], mx4[:], -1.0)
                    qn2b = qn2t[:, T4:T4 + 4].unsqueeze(2).broadcast_to(
                        [128, 4, 8])
                    nc.vector.tensor_tensor(
                        d24[:].rearrange("p (a b) -> p a b", a=4), d24[:]
                        .rearrange("p (a b) -> p a b", a=4), qn2b,
                        op=mybir.AluOpType.add)
                    nc.vector.tensor_scalar_max(d24[:], d24[:], EPS)
                    w84 = work.tile([128, 32], F32, tag="w84")
                    nc.vector.reciprocal(out=w84[:], in_=d24[:])
                    sw4 = small.tile([128, 4], F32, tag="sw4")
                    nc.vector.tensor_reduce(
                        out=sw4[:], in_=w84[:].rearrange(
                            "p (a b) -> p a b", a=4),
                        axis=mybir.AxisListType.X, op=mybir.AluOpType.add)
                    rsw4 = small.tile([128, 4], F32, tag="rsw4")
                    nc.vector.reciprocal(out=rsw4[:], in_=sw4[:])
                    nc.vector.tensor_tensor(
                        wn4[:].rearrange("p (a b) -> p a b", a=4),
                        w84[:].rearrange("p (a b) -> p a b", a=4),
                        rsw4[:].unsqueeze(2).broadcast_to([128, 4, 8]),
                        op=mybir.AluOpType.mult)
                # stage idx: wrapped layout [16, 64] is byte-identical to
                # a row-major copy of mi [128, 8] (thanks to the pi-perm)
                nc.sync.dma_start(
                    out=IDXST[t].rearrange("r (a l) -> r a l", a=8),
                    in_=mi[:])
                # every 4 tiles: transpose wn4 on PE and stage with one DMA.
                # wnT[(t4, l), pi] lands at t4*4096B + l*512B + pi*4B which is
                # exactly WNST[t][p=16l+r][q'] (16*32B == 512B).
                if t % 4 == 3:
                    pwt = ptr.tile([32, 128], F32, name="pwt")
                    nc.tensor.transpose(pwt[:], wn4[:], ident[:])
                    wnT = small.tile([32, 128], F32, tag="wnT")
                    nc.vector.tensor_copy(out=wnT[:], in_=pwt[:])
                    nc.scalar.dma_start(
                        out=WNST[t - 3:t + 1].rearrange(
                            "t (l r) q -> (t l) (r q)", l=8),
                        in_=wnT[:])

            # ---- interp: one gather per query tile (1024 idxs) ----
            # gather list order i = 128*q' + 16*l + r -> G partition 16l+r,
            # sub-chunk q'; query q_local = 16*q' + r
            for t in range(NT):
                idxw = work.tile([128, 64], I16, tag="idxw")
                idx_wr = IDXST[t].bitcast(I16).unsqueeze(0).broadcast_to(
                    [8, 16, 64])
                nc.sync.dma_start(out=idxw[:], in_=idx_wr)
                wnre = work.tile([128, 8], F32, tag="wnre")
                nc.scalar.dma_start(out=wnre[:], in_=WNST[t])
                gt = work.tile([128, 8, C], FP16, tag="G")
                nc.gpsimd.dma_gather(
                    out_ap=gt[:], in_ap=H1D[:], idxs_ap=idxw[:],
                    num_idxs=1024, num_idxs_reg=1024, elem_size=C,
                )
                # wcol[p, c, m] = msk[p, m] * wnre[p, c]
                wcol = work.tile([128, 8, 16], FP16, tag="wcol")
                nc.vector.tensor_tensor(
                    wcol[:],
                    msk[:].unsqueeze(1).broadcast_to([128, 8, 16]),
                    wnre[:].unsqueeze(2).broadcast_to([128, 8, 16]),
                    op=mybir.AluOpType.mult)
                osb = work.tile([16, 8, C], F32, tag="osb")
                for hj in range(2):
                    po = pout.tile([16, 4, C], F32)
                    for cc in range(4):
                        ci = hj * 4 + cc
                        nc.tensor.matmul(
                            po[:, cc, :],
                            wcol[:, ci, :],
                            gt[:, ci, :],
                            start=True, stop=True,
                        )
                    nc.scalar.copy(
                        out=osb[:, hj * 4:(hj + 1) * 4, :], in_=po[:])
                # rows q = 16*c + m  ->  osb[m, c, :]
                nc.sync.dma_start(
                    out=OUT[t * 128:(t + 1) * 128, :]
                    .rearrange("(c m) d -> m c d", m=16),
                    in_=osb[:],
                )

    nc.finalize()
    _NC_CACHE[key] = nc
    return nc


def _prep_core(x1b, pos1b, yb, W1, b1):
    """Host-side layout prep for one core. yb: [NQ, 3] queries."""
    pn2 = (pos1b.astype(np.float32) ** 2).sum(-1).astype(np.float32)
    qn2 = (yb.astype(np.float32) ** 2).sum(-1).astype(np.float32)
    # permute queries within each 128-tile: partition pi holds query
    # 16*(pi%8) + pi//8 (so staging DMAs have contiguous runs)
    pi = np.arange(128)
    q_of_part = 16 * (pi % 8) + pi // 8
    perm = (np.arange(NT)[:, None] * 128 + q_of_part[None, :]).reshape(-1)
    yb = yb[perm]
    qn2 = qn2[perm]
    yaug, paug, _ = _sel_tables(yb, pos1b, pn2)
    import ml_dtypes
    mask16 = np.zeros((128, 16), np.float16)
    for p in range(128):
        mask16[p, p % 16] = 1.0
    return {
        "yaug": np.ascontiguousarray(yaug),
        "paug": np.ascontiguousarray(paug),
        "qn2t": np.ascontiguousarray(qn2.reshape(NT, 128).T),
        "x1t": np.ascontiguousarray(x1b.T.reshape(2, 128, N1)),
        "w1t": np.ascontiguousarray(W1.T.reshape(2, 128, C)),
        "b1rep": np.ascontiguousarray(np.broadcast_to(b1, (128, C))),
        "mask16": mask16,
    }


def kernel(x1, pos1, x2, pos2, W1, b1, W2, b2):
    x1 = np.asarray(x1, np.float32)
    pos1 = np.asarray(pos1, np.float32)
    pos2 = np.asarray(pos2, np.float32)
    W1 = np.asarray(W1, np.float32)
    b1 = np.asarray(b1, np.float32)

    _, _, sel_dt = _sel_tables(pos2[0, :8, :], pos1[0, :8, :],
                               np.zeros(8, np.float32))
    sel_k = {"fp32": 4}.get(SEL_MODE, 21)
    nc = build_nc(sel_k, sel_dt)

    in_maps = []
    for c in range(8):
        b, h = c // 2, c % 2
        yb = pos2[b, h * NQ:(h + 1) * NQ]
        in_maps.append(_prep_core(x1[b], pos1[b], yb, W1, b1))

    import os
    trace = os.environ.get("KNN_TRACE", "0") == "1"
    if trace:
        import types, ctypes, contextlib
        import concourse.bass_utils as bu
        bu.upload_artifacts = lambda d: str(d)
        lib = ctypes.CDLL("/opt/axon/libaxon_pjrt.so")
        lib.axon_start_nrt_profile.argtypes = [
            ctypes.POINTER(ctypes.c_int64), ctypes.c_size_t]
        lib.axon_start_nrt_profile.restype = ctypes.c_int64
        lib.axon_stop_nrt_profile.argtypes = [ctypes.c_char_p]
        lib.axon_stop_nrt_profile.restype = ctypes.c_int64

        @contextlib.contextmanager
        def _hook(output_dir, device_ids):
            import jax
            jax.devices()
            if device_ids:
                ids = (ctypes.c_int64 * len(device_ids))(*device_ids)
                rc = lib.axon_start_nrt_profile(ids, len(device_ids))
            else:
                rc = lib.axon_start_nrt_profile(None, 0)
            if rc != 0:
                raise RuntimeError(f"axon_start_nrt_profile rc={rc}")
            try:
                yield
            finally:
                n = lib.axon_stop_nrt_profile(str(output_dir).encode())
                print(f"profile: {n} ntff file(s) written to {output_dir}")

        mod = types.ModuleType("antenv.axon_hooks")
        mod.get_axon_ntff_profile_hook = lambda: (
            lambda output_dir, device_ids: _hook(output_dir, device_ids))
        sys.modules["antenv.axon_hooks"] = mod
        res = run_bass_kernel_spmd(
            nc, in_maps, list(range(8)), trace=True,
            tmpdir=os.environ.get("KNN_TRACE_DIR") or None)
        print("HW exec time:", res.exec_time_ns, "ns")
        if res.mean_exec_time_ns:
            print("mean exec time:", res.mean_exec_time_ns, "ns")
    else:
        res = run_bass_kernel_spmd(nc, in_maps, list(range(8)))

    out = np.empty((B, N2, C), np.float32)
    for c in range(8):
        b, h = c // 2, c % 2
        out[b, h * NQ:(h + 1) * NQ] = res.results[c]["out"]
    return out


if __name__ == "__main__":
    rng = np.random.default_rng(0)
    ins = {
        "x1": rng.standard_normal((B, N1, C), np.float32),
        "pos1": rng.random((B, N1, 3), np.float32),
        "x2": rng.standard_normal((B, N2, C), np.float32),
        "pos2": rng.random((B, N2, 3), np.float32),
        "W1": (rng.standard_normal((C, C), np.float32) * 0.05).astype(np.float32),
        "b1": (rng.standard_normal((C,), np.float32) * 0.05).astype(np.float32),
        "W2": rng.standard_normal((C, C), np.float32),
        "b2": rng.standard_normal((C,), np.float32),
    }
    o = kernel(**ins)
    print("kernel out", o.shape, o.dtype, np.abs(o).max())
